# revision 1
# baseline (speedup 1.0000x reference)
"""GNN message-passing encoder (nn_Encoder_52252572123266) on 8 TRN2 NeuronCores.

Strategy: receiver-range edge sharding. Core k owns nodes [2500k, 2500(k+1))
and every edge whose receiver lies in that range (~40k edges each for uniform
receivers). The segment_sum therefore needs no cross-core reduction; only the
updated node features x are AllGathered (1.25 MB/rank) between rounds.

On-core layout: activations are feature-major ([D on partitions, rows on the
free dim]) so MLPs chain as lhsT=W (stationary), rhs=activation (moving, 512
rows/chunk), with fp32r matmuls (full PE speed, ~1e-4 rounding). Gathers use
the ANT dma_gather custom DMA (row gather from HBM, edge-major) followed by
PE transposes back to feature-major. segment_sum is a one-hot matmul: edges
sorted by receiver into 512-node windows, sel[e, n] = (recv_rel[e] == n)
built on DVE via iota compare, agg accumulated in PSUM per window.
"""

import math
import os
from contextlib import ExitStack

import numpy as np

import concourse.bass as bass  # noqa: F401  (import keeps bass registered)
import concourse.tile as tile
from concourse import bacc, mybir
from concourse.bass_utils import run_bass_kernel_spmd

P = 128
N_CORES = 8
N = 20000
E = 320000
D = 128
H = 256
NUM_FINE = 2

NODES_PER_CORE = N // N_CORES            # 2500
WIN = 256                                 # nodes per scatter window
N_WIN = math.ceil(NODES_PER_CORE / WIN)   # 5
NODE_SLOTS = N_WIN * WIN                  # 2560 padded node slots per core
GATHER_MACROS = 2                         # macros per dma_gather call
GB = GATHER_MACROS * 512                  # idxs per gather call

F32 = mybir.dt.float32
F32R = mybir.dt.float32r
I16 = mybir.dt.int16
RELU = mybir.ActivationFunctionType.Relu


# ----------------------------------------------------------------------------
# Host-side preparation
# ----------------------------------------------------------------------------

def _pad_slot(ids):
    """global node id -> padded-global slot id (core-major, NODE_SLOTS per core)."""
    return NODE_SLOTS * (ids // NODES_PER_CORE) + ids % NODES_PER_CORE


def _wrap_idx16(slots, ep_g):
    """Pack indices into the ANT dma_gather layout: [128, ep_g/16] int16 with
    idx[i] at [i%16, i//16], replicated across the 8 groups of 16 partitions."""
    flat = np.zeros(ep_g, dtype=np.int16)
    flat[: len(slots)] = slots.astype(np.int16)
    a = flat.reshape(ep_g // 16, 16).T
    return np.ascontiguousarray(np.tile(a, (8, 1)))


def prepare_core(k, nodes, edges, senders, receivers, ep_win, ep_g):
    """Build the per-core input arrays for core k."""
    lo = k * NODES_PER_CORE
    hi = lo + NODES_PER_CORE
    eids = np.nonzero((receivers >= lo) & (receivers < hi))[0]
    rloc = receivers[eids] - lo
    order = np.argsort(rloc, kind="stable")
    eids = eids[order]
    rloc = rloc[order]

    ep = N_WIN * ep_win
    w = rloc // WIN
    send_slots = np.zeros(ep, dtype=np.int64)        # pad -> 0
    recv_slots = np.zeros(ep, dtype=np.int64)        # pad -> 0
    recv_rel = np.full(ep, -1.0, dtype=np.float32)   # pad -> -1 (sel row = 0)
    perm = np.full(ep, -1, dtype=np.int64)           # stream pos -> edge id
    for wi in range(N_WIN):
        sel = w == wi
        cnt = int(sel.sum())
        assert cnt <= ep_win, (k, wi, cnt, ep_win)
        base = wi * ep_win
        perm[base : base + cnt] = eids[sel]
        send_slots[base : base + cnt] = _pad_slot(senders[eids[sel]])
        recv_slots[base : base + cnt] = _pad_slot(receivers[eids[sel]])
        recv_rel[base : base + cnt] = (rloc[sel] - wi * WIN).astype(np.float32)

    edges_T = np.zeros((D, ep), dtype=np.float32)
    real = perm >= 0
    edges_T[:, real] = edges[perm[real]].T

    nodes_T = np.zeros((D, NODE_SLOTS), dtype=np.float32)
    nodes_T[:, :NODES_PER_CORE] = nodes[lo:hi].T

    return dict(
        edges_T=edges_T,
        nodes_T=nodes_T,
        send_idx=_wrap_idx16(send_slots, ep_g),
        recv_idx=_wrap_idx16(recv_slots, ep_g),
        recv_rel=np.ascontiguousarray(recv_rel.reshape(ep // P, P).T),
    )


def build_weight_blob(ws):
    """Concatenate weight k-tile blocks + identity + ones into one
    (128, WCOLS) fp32 array. Returns (blob, {name: (col, M)})."""
    cols = []
    offs = {}
    c = 0
    for name, wmat in ws.items():
        K, M = wmat.shape
        for kt in range(K // P):
            cols.append(np.asarray(wmat[kt * P : (kt + 1) * P, :], dtype=np.float32))
        offs[name] = (c, M)
        c += (K // P) * M
    offs["ident"] = (c, P)
    cols.append(np.eye(P, dtype=np.float32))
    c += P
    offs["ones"] = (c, 1)
    cols.append(np.ones((P, 1), dtype=np.float32))
    c += 1
    return np.concatenate(cols, axis=1), offs


def build_aux_blob(bs):
    """Biases (one [128,1] col per m-tile) + iota row -> (128, cols) fp32."""
    cols = []
    offs = {}
    c = 0
    for name, b in bs.items():
        b = np.asarray(b, dtype=np.float32)
        nmt = len(b) // P
        cols.append(b.reshape(nmt, P).T)
        offs[name] = c
        c += nmt
    offs["iota"] = c
    cols.append(np.tile(np.arange(WIN, dtype=np.float32)[None, :], (P, 1)))
    c += WIN
    return np.concatenate(cols, axis=1), offs


# ----------------------------------------------------------------------------
# Bass program
# ----------------------------------------------------------------------------

def build_program(ep_win, ep_g, wcols, acols, woffs, aoffs):
    ep = N_WIN * ep_win
    n_macro_w = ep_win // 512
    n_macro = ep // 512
    n_nchunk = NODE_SLOTS // 512

    nc = bacc.Bacc(None, target_bir_lowering=False, debug=False)

    edges_T = nc.dram_tensor("edges_T", [P, ep], F32, kind="ExternalInput")
    nodes_T = nc.dram_tensor("nodes_T", [P, NODE_SLOTS], F32, kind="ExternalInput")
    send_idx = nc.dram_tensor("send_idx", [P, ep_g // 16], I16, kind="ExternalInput")
    recv_idx = nc.dram_tensor("recv_idx", [P, ep_g // 16], I16, kind="ExternalInput")
    recv_rel = nc.dram_tensor("recv_rel", [P, ep // P], F32, kind="ExternalInput")
    wblob = nc.dram_tensor("wblob", [P, wcols], F32, kind="ExternalInput")
    ablob = nc.dram_tensor("ablob", [P, acols], F32, kind="ExternalInput")
    out_pooled = nc.dram_tensor(
        "out_pooled", [NUM_FINE, NODE_SLOTS], F32, kind="ExternalOutput"
    )

    with tile.TileContext(nc) as tc, ExitStack() as ctx:
        sb1 = ctx.enter_context(tc.tile_pool(name="sb1", bufs=1))
        dram = ctx.enter_context(tc.tile_pool(name="dram", bufs=1, space="DRAM"))
        pml = ctx.enter_context(tc.tile_pool(name="pml", bufs=6))
        pg = ctx.enter_context(tc.tile_pool(name="pg", bufs=3))
        pxf = ctx.enter_context(tc.tile_pool(name="pxf", bufs=4))
        pxnm = ctx.enter_context(tc.tile_pool(name="pxnm", bufs=1))
        pxo = ctx.enter_context(tc.tile_pool(name="pxo", bufs=2))
        pagg = ctx.enter_context(tc.tile_pool(name="pagg", bufs=1))
        psel = ctx.enter_context(tc.tile_pool(name="psel", bufs=3))
        pout = ctx.enter_context(tc.tile_pool(name="pout", bufs=1))
        ph = ctx.enter_context(tc.tile_pool(name="ph", bufs=3, space="PSUM"))
        pe_ps = ctx.enter_context(tc.tile_pool(name="pe_ps", bufs=2, space="PSUM"))
        ptr = ctx.enter_context(tc.tile_pool(name="ptr", bufs=2, space="PSUM"))
        pag_ps = ctx.enter_context(tc.tile_pool(name="pag_ps", bufs=1, space="PSUM"))

        # resident tiles
        wsb = sb1.tile([P, wcols], F32R)
        nc.gpsimd.dma_start(wsb[:], wblob[:].bitcast(F32R))
        asb = sb1.tile([P, acols], F32)
        nc.gpsimd.dma_start(asb[:], ablob[:])
        sidx = sb1.tile([P, ep_g // 16], I16)
        nc.gpsimd.dma_start(sidx[:], send_idx[:])
        ridx = sb1.tile([P, ep_g // 16], I16)
        nc.gpsimd.dma_start(ridx[:], recv_idx[:])
        rrel = sb1.tile([P, ep // P], F32)
        nc.gpsimd.dma_start(rrel[:], recv_rel[:])

        def w_ap(name, kt):
            c, m = woffs[name]
            return wsb[:, c + kt * m : c + (kt + 1) * m]

        ident = w_ap("ident", 0)
        ones_col = w_ap("ones", 0)

        def b_ap(name, mt):
            c = aoffs[name]
            return asb[:, c + mt : c + mt + 1]

        iota_f = asb[:, aoffs["iota"] : aoffs["iota"] + WIN]

        # DRAM intermediates
        e_a = dram.tile([P, ep], F32, tag="ea")
        e_b = dram.tile([P, ep], F32, tag="eb")
        x_pad = [dram.tile([NODE_SLOTS, P], F32, name=f"xpad{t}", tag=f"xpad{t}") for t in range(2)]
        x_full = [
            dram.tile([N_CORES * NODE_SLOTS, P], F32, name=f"xfull{t}", tag=f"xfull{t}", addr_space=("Local" if os.environ.get("KNOCC") == "1" else "Shared"))
            for t in range(2)
        ]

        # PE warmup ladder: absorb fresh semaphores one at a time.
        wu = ptr.tile([P, P], F32R, tag="tr")
        nc.tensor.transpose(wu[:], ident, ident)

        def relu_to(engine, dst_ap, src_ap, bias):
            if engine == "act":
                nc.scalar.activation(dst_ap, src_ap, RELU, bias=bias)
            else:
                nc.vector.tensor_scalar(
                    out=dst_ap,
                    in0=src_ap,
                    scalar1=bias,
                    scalar2=0.0,
                    op0=mybir.AluOpType.add,
                    op1=mybir.AluOpType.max,
                )

        def mlp2(rhs_list, w1, b1, w2, b2, out_ap, engines=("act", "dve", "act")):
            """Two-layer MLP on one 512-col chunk (feature-major)."""
            nmt1 = woffs[w1][1] // P
            hts = []
            for mt in range(nmt1):
                hp = ph.tile([P, 512], F32, tag="h")
                for kt, rhs in enumerate(rhs_list):
                    nc.tensor.matmul(
                        hp[:],
                        w_ap(w1, kt)[:, mt * P : (mt + 1) * P],
                        rhs,
                        start=(kt == 0),
                        stop=(kt == len(rhs_list) - 1),
                    )
                ht = pml.tile([P, 512], F32R, tag="hsb")
                relu_to(engines[mt % 2], ht[:], hp[:], b_ap(b1, mt))
                hts.append(ht)
            ep2 = pe_ps.tile([P, 512], F32, tag="eps")
            for kt in range(nmt1):
                nc.tensor.matmul(
                    ep2[:],
                    w_ap(w2, kt),
                    hts[kt][:],
                    start=(kt == 0),
                    stop=(kt == nmt1 - 1),
                )
            relu_to(engines[2], out_ap, ep2[:], b_ap(b2, 0))

        _tr_tick = [0]

        def transpose128(src_ap, dst_ap):
            tp = ptr.tile([P, P], F32R, tag="tr")
            nc.tensor.transpose(tp[:], src_ap, ident)
            _tr_tick[0] ^= 1
            if _tr_tick[0]:
                nc.vector.tensor_copy(dst_ap, tp[:])
            else:
                nc.scalar.copy(dst_ap, tp[:])

        # ---------------- fine iterations ----------------
        pooled_sb = [
            pout.tile([1, NODE_SLOTS], F32, name=f"pool{t}", tag=f"pool{t}")
            for t in range(NUM_FINE)
        ]
        for t in range(NUM_FINE):
            nc.gpsimd.memset(pooled_sb[t][:], 0.0)

        KREPEAT = int(os.environ.get("KREPEAT", "1"))
        for _rep in range(KREPEAT):
            # ---------------- node embed ----------------
            x_own = pxo.tile([P, NODE_SLOTS], F32R, tag="xo")
            for nch in range(n_nchunk):
                nt = pml.tile([P, 512], F32R, tag="in512")
                nc.sync.dma_start(
                    nt[:], nodes_T[:, nch * 512 : (nch + 1) * 512].bitcast(F32R)
                )
                mlp2([nt[:]], "Wn1", "bn1", "Wn2", "bn2",
                     x_own[:, nch * 512 : (nch + 1) * 512])

            def allgather_x(x_own_t, t):
                xnm = pxnm.tile([P, NODE_SLOTS], F32R, tag="xnm")
                for c in range(NODE_SLOTS // P):
                    transpose128(
                        x_own_t[:, c * P : (c + 1) * P], xnm[:, c * P : (c + 1) * P]
                    )
                nc.sync.dma_start(
                    x_pad[t][:].rearrange("(c p) f -> p c f", p=P).bitcast(F32R),
                    xnm[:].rearrange("p (c f) -> p c f", f=P),
                )
                if os.environ.get("KNOCC") == "1":
                    # TimelineSim-able stand-in: copy own block only
                    nc.gpsimd.dma_start(x_full[t][0:NODE_SLOTS, :], x_pad[t][:])
                else:
                    nc.gpsimd.collective_compute(
                        "AllGather",
                        mybir.AluOpType.bypass,
                        ins=[x_pad[t].opt()],
                        outs=[x_full[t].opt()],
                        replica_groups=[list(range(N_CORES))],
                    )

            allgather_x(x_own, 0)

            # ---------------- edge embed ----------------
            for mc in range(n_macro):
                et = pml.tile([P, 512], F32R, tag="in512")
                nc.sync.dma_start(
                    et[:], edges_T[:, mc * 512 : (mc + 1) * 512].bitcast(F32R)
                )
                e_out = pml.tile([P, 512], F32R, tag="eo")
                mlp2([et[:]], "We1", "be1", "We2", "be2", e_out[:])
                nc.sync.dma_start(
                    e_a[:, mc * 512 : (mc + 1) * 512].bitcast(F32R), e_out[:]
                )

            STAGES = int(os.environ.get("KSTAGES", str(NUM_FINE)))
            for t in range(STAGES):
                e_in = e_a if t == 0 else e_b
                xf = x_full[t]
                agg_sb = pagg.tile([P, NODE_SLOTS], F32R, tag="agg")
                g_s = g_r = None

                for wi in range(N_WIN):
                    agg_ps = pag_ps.tile([P, WIN], F32, tag="aggps")
                    for mcw in range(n_macro_w):
                        mc = wi * n_macro_w + mcw
                        gi = mc % GATHER_MACROS
                        if gi == 0:
                            g_s = pg.tile([P, GATHER_MACROS * 4, P], F32R, tag="gs")
                            g_r = pg.tile([P, GATHER_MACROS * 4, P], F32R, tag="gr")
                            i0 = mc * 512 // 16
                            i1 = i0 + GB // 16
                            if os.environ.get("KNOGATHER") == "1":
                                fake = xf[0:GB, :].rearrange("(c p) f -> p c f", p=P)
                                nc.gpsimd.dma_start(g_s[:], fake.bitcast(F32R))
                                nc.gpsimd.dma_start(g_r[:], fake.bitcast(F32R))
                            else:
                                nc.gpsimd.dma_gather(
                                    out_ap=g_s[:],
                                    in_ap=xf[:].bitcast(F32R),
                                    idxs_ap=sidx[:, i0:i1],
                                    num_idxs=GB,
                                    num_idxs_reg=GB,
                                    elem_size=P,
                                    queue_num=0,
                                )
                                nc.gpsimd.dma_gather(
                                    out_ap=g_r[:],
                                    in_ap=xf[:].bitcast(F32R),
                                    idxs_ap=ridx[:, i0:i1],
                                    num_idxs=GB,
                                    num_idxs_reg=GB,
                                    elem_size=P,
                                    queue_num=0,
                                )
                        ec = pml.tile([P, 512], F32R, tag="in512")
                        nc.sync.dma_start(
                            ec[:], e_in[:, mc * 512 : (mc + 1) * 512].bitcast(F32R)
                        )
                        xs_fm = pxf.tile([P, 512], F32R, tag="xsfm")
                        xr_fm = pxf.tile([P, 512], F32R, tag="xrfm")
                        for j in range(4):
                            transpose128(g_s[:, gi * 4 + j, :], xs_fm[:, j * P : (j + 1) * P])
                            transpose128(g_r[:, gi * 4 + j, :], xr_fm[:, j * P : (j + 1) * P])
                        e_new = pml.tile([P, 512], F32R, tag="eo")
                        mlp2(
                            [xs_fm[:], xr_fm[:], ec[:]],
                            "Wed1", "bed1", "Wed2", "bed2", e_new[:],
                        )
                        if t < NUM_FINE - 1:
                            nc.sync.dma_start(
                                e_b[:, mc * 512 : (mc + 1) * 512].bitcast(F32R), e_new[:]
                            )
                        for j in range(4):
                            sc = mc * 4 + j
                            eT = pxf.tile([P, P], F32R, tag="eT")
                            transpose128(e_new[:, j * P : (j + 1) * P], eT[:])
                            selt = psel.tile([P, WIN], F32R, tag="sel")
                            nc.vector.tensor_tensor(
                                out=selt[:],
                                in0=rrel[:, sc : sc + 1].to_broadcast([P, WIN]),
                                in1=iota_f,
                                op=mybir.AluOpType.is_equal,
                            )
                            nc.tensor.matmul(
                                agg_ps[:],
                                eT[:],
                                selt[:],
                                start=(mcw == 0 and j == 0),
                                stop=(mcw == n_macro_w - 1 and j == 3),
                                skip_group_check=True,
                            )
                    nc.vector.tensor_copy(agg_sb[:, wi * WIN : (wi + 1) * WIN], agg_ps[:])

                # node MLP + pooled
                x_new = pxo.tile([P, NODE_SLOTS], F32R, tag="xo")
                for ncn in range(n_nchunk):
                    sl = slice(ncn * 512, (ncn + 1) * 512)
                    mlp2(
                        [x_own[:, sl], agg_sb[:, sl]],
                        "Wnd1", "bnd1", "Wnd2", "bnd2", x_new[:, sl],
                    )
                    pp = pe_ps.tile([1, 512], F32, tag="eps")
                    nc.tensor.matmul(pp[:], ones_col, x_new[:, sl], start=True, stop=True)
                    nc.vector.tensor_copy(pooled_sb[t][:, sl], pp[:])
                x_own = x_new
                if t < NUM_FINE - 1:
                    allgather_x(x_own, t + 1)

        for t in range(NUM_FINE):
            nc.sync.dma_start(out_pooled[t : t + 1, :], pooled_sb[t][:])

    nc.compile()
    return nc


# ----------------------------------------------------------------------------
# Entry point
# ----------------------------------------------------------------------------

def _prepare(inputs):
    nodes = np.asarray(inputs["nodes"], dtype=np.float32)
    edges = np.asarray(inputs["edges"], dtype=np.float32)
    senders = np.asarray(inputs["senders"]).astype(np.int64)
    receivers = np.asarray(inputs["receivers"]).astype(np.int64)

    ws = {k: np.asarray(inputs[k]) for k in
          ["Wn1", "Wn2", "We1", "We2", "Wed1", "Wed2", "Wnd1", "Wnd2"]}
    bs = {k: np.asarray(inputs[k]) for k in
          ["bn1", "bn2", "be1", "be2", "bed1", "bed2", "bnd1", "bnd2"]}

    rslot = _pad_slot(receivers)
    counts = np.bincount(rslot // WIN, minlength=N_CORES * N_WIN)
    ep_win = int(math.ceil(counts.max() / 512) * 512)
    ep = N_WIN * ep_win
    ep_g = int(math.ceil(ep / GB) * GB)

    wblob, woffs = build_weight_blob(ws)
    ablob, aoffs = build_aux_blob(bs)

    in_maps = []
    for k in range(N_CORES):
        m = prepare_core(k, nodes, edges, senders, receivers, ep_win, ep_g)
        m["wblob"] = wblob
        m["ablob"] = ablob
        in_maps.append(m)

    nc = build_program(ep_win, ep_g, wblob.shape[1], ablob.shape[1], woffs, aoffs)
    return nc, in_maps


def _assemble(results):
    out = np.zeros(NUM_FINE * N, dtype=np.float32)
    for k in range(N_CORES):
        pooled = results[k]["out_pooled"]
        for t in range(NUM_FINE):
            out[t * N + k * NODES_PER_CORE : t * N + (k + 1) * NODES_PER_CORE] = (
                pooled[t, :NODES_PER_CORE]
            )
    return out


def _run(inputs, trace=False):
    nc, in_maps = _prepare(inputs)
    res = run_bass_kernel_spmd(
        nc, in_maps, core_ids=list(range(N_CORES)), trace=trace
    )
    return _assemble(res.results), res


def kernel(**inputs):
    out, _ = _run(inputs, trace=False)
    return out



# revision 10
# speedup vs baseline: 2.0255x; 2.0255x over previous
"""GNN message-passing encoder (nn_Encoder_52252572123266) on 8 TRN2 NeuronCores.

Strategy: receiver-range edge sharding. Core k owns nodes [2500k, 2500(k+1))
and every edge whose receiver lies in that range (~40k edges each). The
segment_sum needs no cross-core reduction; only the updated node features x
are AllGathered (0.65 MB/rank bf16) between rounds.

v2 layout: everything bf16 on-chip; edge features live in SBUF for the whole
kernel (no HBM edge traffic after the initial load). Sender features are
gathered from the AllGathered x_full in HBM via the ANT dma_gather in
transpose mode (bf16 rows land feature-major, no PE transposes). Receiver
features are folded into the edge-MLP layer 1 via a windowed one-hot matmul:
YrT = x_win^T @ W1r per 256-node window, then h += YrT.T @ sel where
sel[n,e] = (recv_rel[e] == n) is built on DVE from a partition-replicated
copy of recv_rel streamed from HBM. The segment_sum stays a one-hot matmul
(edge-major sel built from iota compare), accumulated in PSUM per window.
"""

import math
import os
from contextlib import ExitStack

import numpy as np
import ml_dtypes

import concourse.bass as bass  # noqa: F401  (import keeps bass registered)
import concourse.tile as tile
from concourse import bacc, mybir
from concourse.bass_utils import run_bass_kernel_spmd

P = 128
N_CORES = 8
N = 20000
E = 320000
D = 128
H = 256
NUM_FINE = 2

NODES_PER_CORE = N // N_CORES            # 2500
WIN = 256                                 # nodes per scatter window
N_WIN = math.ceil(NODES_PER_CORE / WIN)   # 10
NODE_SLOTS = N_WIN * WIN                  # 2560 padded node slots per core
GB = 1024                                 # idxs per dma_gather call
GATHER_T = os.environ.get("KGATHER_T", "0") == "1"  # transpose-mode dma_gather

F32 = mybir.dt.float32
BF16 = mybir.dt.bfloat16
I16 = mybir.dt.int16
RELU = mybir.ActivationFunctionType.Relu
BF = ml_dtypes.bfloat16


# ----------------------------------------------------------------------------
# Host-side preparation
# ----------------------------------------------------------------------------

def _pad_slot(ids):
    """global node id -> padded-global slot id (core-major, NODE_SLOTS per core)."""
    return NODE_SLOTS * (ids // NODES_PER_CORE) + ids % NODES_PER_CORE


def _wrap_idx16(slots, ep):
    """Pack indices into the ANT dma_gather layout: [128, ep/16] int16 with
    idx[i] at [i%16, i//16], replicated across the 8 groups of 16 partitions."""
    flat = np.zeros(ep, dtype=np.int16)
    flat[: len(slots)] = slots.astype(np.int16)
    a = flat.reshape(ep // 16, 16).T
    return np.ascontiguousarray(np.tile(a, (8, 1)))


def prepare_core(k, nodes, edges, senders, receivers, ep_win):
    """Build the per-core input arrays for core k."""
    lo = k * NODES_PER_CORE
    hi = lo + NODES_PER_CORE
    eids = np.nonzero((receivers >= lo) & (receivers < hi))[0]
    rloc = receivers[eids] - lo
    order = np.argsort(rloc, kind="stable")
    eids = eids[order]
    rloc = rloc[order]

    ep = N_WIN * ep_win
    w = rloc // WIN
    send_slots = np.zeros(ep, dtype=np.int64)        # pad -> 0
    recv_rel = np.full(ep, -1.0, dtype=np.float32)   # pad -> -1 (sel row = 0)
    perm = np.full(ep, -1, dtype=np.int64)           # stream pos -> edge id
    for wi in range(N_WIN):
        sel = w == wi
        cnt = int(sel.sum())
        assert cnt <= ep_win, (k, wi, cnt, ep_win)
        base = wi * ep_win
        perm[base : base + cnt] = eids[sel]
        send_slots[base : base + cnt] = _pad_slot(senders[eids[sel]])
        recv_rel[base : base + cnt] = (rloc[sel] - wi * WIN).astype(np.float32)

    edges_T = np.zeros((D, ep), dtype=BF)
    real = perm >= 0
    edges_T[:, real] = edges[perm[real]].T.astype(BF)

    nodes_T = np.zeros((D, NODE_SLOTS), dtype=BF)
    nodes_T[:, :NODES_PER_CORE] = nodes[lo:hi].T.astype(BF)

    rr16 = recv_rel.astype(BF)
    return dict(
        edges_T=edges_T,
        nodes_T=nodes_T,
        send_idx=_wrap_idx16(send_slots, ep),
        # edge-major column layout: [p, sc] = recv_rel[sc*128 + p]
        rrel_col=np.ascontiguousarray(rr16.reshape(ep // P, P).T),
        # partition-replicated: [p, e] = recv_rel[e]
        rrel_rep=np.ascontiguousarray(np.tile(rr16[None, :], (P, 1))),
    )


def build_weight_blob(ws):
    """Concatenate weight k-tile blocks + identity + ones into one
    (128, WCOLS) bf16 array. Returns (blob, {name: (col, M)})."""
    cols = []
    offs = {}
    c = 0
    for name, wmat in ws.items():
        K, M = wmat.shape
        for kt in range(K // P):
            cols.append(np.asarray(wmat[kt * P : (kt + 1) * P, :], dtype=BF))
        offs[name] = (c, M)
        c += (K // P) * M
    offs["ident"] = (c, P)
    cols.append(np.eye(P, dtype=BF))
    c += P
    offs["ones"] = (c, 1)
    cols.append(np.ones((P, 1), dtype=BF))
    c += 1
    offs["iota"] = (c, WIN)
    cols.append(np.tile(np.arange(WIN, dtype=np.float32)[None, :], (P, 1)).astype(BF))
    c += WIN
    offs["icol"] = (c, 2)
    icol = np.stack([np.arange(P), np.arange(P) + P], axis=1).astype(np.float32)
    cols.append(icol.astype(BF))
    c += 2
    return np.concatenate(cols, axis=1), offs


def build_bias_blob(bs):
    """Biases (one [128,1] col per m-tile) -> (128, cols) fp32."""
    cols = []
    offs = {}
    c = 0
    for name, b in bs.items():
        b = np.asarray(b, dtype=np.float32)
        nmt = len(b) // P
        cols.append(b.reshape(nmt, P).T)
        offs[name] = c
        c += nmt
    return np.concatenate(cols, axis=1), offs


# ----------------------------------------------------------------------------
# Bass program
# ----------------------------------------------------------------------------

def build_program(ep_win, wcols, bcols, woffs, boffs):
    ep = N_WIN * ep_win
    n_macro_w = ep_win // 512
    n_macro = ep // 512
    n_nchunk = NODE_SLOTS // 512
    knocc = os.environ.get("KNOCC") == "1"

    nc = bacc.Bacc(None, target_bir_lowering=False, debug=False,
                   num_swdge_queues=4)

    edges_T = nc.dram_tensor("edges_T", [P, ep], BF16, kind="ExternalInput")
    nodes_T = nc.dram_tensor("nodes_T", [P, NODE_SLOTS], BF16, kind="ExternalInput")
    send_idx = nc.dram_tensor("send_idx", [P, ep // 16], I16, kind="ExternalInput")
    rrel_col = nc.dram_tensor("rrel_col", [P, ep // P], BF16, kind="ExternalInput")
    rrel_rep = nc.dram_tensor("rrel_rep", [P, ep], BF16, kind="ExternalInput")
    wblob = nc.dram_tensor("wblob", [P, wcols], BF16, kind="ExternalInput")
    bblob = nc.dram_tensor("bblob", [P, bcols], F32, kind="ExternalInput")
    out_pooled = nc.dram_tensor(
        "out_pooled", [NUM_FINE, NODE_SLOTS], F32, kind="ExternalOutput"
    )

    with tile.TileContext(nc) as tc, ExitStack() as ctx:
        sb1 = ctx.enter_context(tc.tile_pool(name="sb1", bufs=1))
        dram = ctx.enter_context(tc.tile_pool(name="dram", bufs=1, space="DRAM"))
        pml = ctx.enter_context(tc.tile_pool(name="pml", bufs=6))
        pg = ctx.enter_context(tc.tile_pool(name="pg", bufs=3))
        prr = ctx.enter_context(tc.tile_pool(name="prr", bufs=3))
        psel = ctx.enter_context(tc.tile_pool(name="psel", bufs=3))
        pyr = ctx.enter_context(tc.tile_pool(name="pyr", bufs=2))
        pxnm = ctx.enter_context(tc.tile_pool(name="pxnm", bufs=1))
        pxo = ctx.enter_context(tc.tile_pool(name="pxo", bufs=2))
        pagg = ctx.enter_context(tc.tile_pool(name="pagg", bufs=1))
        pout = ctx.enter_context(tc.tile_pool(name="pout", bufs=1))
        ph = ctx.enter_context(tc.tile_pool(name="ph", bufs=2, space="PSUM"))
        pe_ps = ctx.enter_context(tc.tile_pool(name="pe_ps", bufs=2, space="PSUM"))
        ptr = ctx.enter_context(tc.tile_pool(name="ptr", bufs=1, space="PSUM"))
        pyr_ps = ctx.enter_context(tc.tile_pool(name="pyr_ps", bufs=1, space="PSUM"))
        pag_ps = ctx.enter_context(tc.tile_pool(name="pag_ps", bufs=1, space="PSUM"))

        # resident tiles
        wsb = sb1.tile([P, wcols], BF16)
        nc.gpsimd.dma_start(wsb[:], wblob[:])
        bsb = sb1.tile([P, bcols], F32)
        nc.gpsimd.dma_start(bsb[:], bblob[:])
        sidx = sb1.tile([P, ep // 16], I16)
        nc.gpsimd.dma_start(sidx[:], send_idx[:])
        rrelc = sb1.tile([P, ep // P], BF16)
        nc.gpsimd.dma_start(rrelc[:], rrel_col[:])
        efeat = sb1.tile([P, ep], BF16)

        def w_ap(name, kt):
            c, m = woffs[name]
            return wsb[:, c + kt * m : c + (kt + 1) * m]

        ident = w_ap("ident", 0)
        ones_col = w_ap("ones", 0)
        iota_f = w_ap("iota", 0)
        icol = w_ap("icol", 0)

        def b_ap(name, mt):
            c = boffs[name]
            return bsb[:, c + mt : c + mt + 1]

        # DRAM intermediates (node-major x for gather + collective)
        x_pad = [dram.tile([NODE_SLOTS, P], BF16, name=f"xpad{t}", tag=f"xpad{t}")
                 for t in range(2)]
        x_full = [
            dram.tile([N_CORES * NODE_SLOTS, P], BF16, name=f"xfull{t}",
                      tag=f"xfull{t}",
                      addr_space=("Local" if knocc else "Shared"))
            for t in range(2)
        ]

        # PE warmup ladder: absorb fresh semaphores one at a time.
        wu = ptr.tile([P, 512], BF16, tag="xtr")
        nc.tensor.transpose(wu[:, 0:P], ident, ident)

        def relu_to(engine, dst_ap, src_ap, bias):
            if engine == "act":
                nc.scalar.activation(dst_ap, src_ap, RELU, bias=bias)
            else:
                nc.vector.tensor_scalar(
                    out=dst_ap,
                    in0=src_ap,
                    scalar1=bias,
                    scalar2=0.0,
                    op0=mybir.AluOpType.add,
                    op1=mybir.AluOpType.max,
                )

        def mlp2(rhs_list, w1, b1, w2, b2, out_ap, engines=("act", "dve", "act"),
                 extra_l1=None):
            """Two-layer MLP on one 512-col chunk (feature-major, bf16).

            extra_l1: list of (lhsT_by_mt, rhs) appended to the layer-1
            accumulation (e.g. the one-hot receiver term)."""
            nmt1 = woffs[w1][1] // P
            n_extra = len(extra_l1) if extra_l1 else 0
            hts = []
            for mt in range(nmt1):
                hp = ph.tile([P, 512], F32, tag="h")
                n_terms = len(rhs_list) + n_extra
                ti = 0
                for kt, rhs in enumerate(rhs_list):
                    nc.tensor.matmul(
                        hp[:],
                        w_ap(w1, kt)[:, mt * P : (mt + 1) * P],
                        rhs,
                        start=(ti == 0),
                        stop=(ti == n_terms - 1),
                        skip_group_check=True,
                    )
                    ti += 1
                if extra_l1:
                    for lhsT_by_mt, rhs in extra_l1:
                        nc.tensor.matmul(
                            hp[:],
                            lhsT_by_mt(mt),
                            rhs,
                            start=(ti == 0),
                            stop=(ti == n_terms - 1),
                            skip_group_check=True,
                        )
                        ti += 1
                ht = pml.tile([P, 512], BF16, tag="hsb")
                relu_to(engines[mt % 2], ht[:], hp[:], b_ap(b1, mt))
                hts.append(ht)
            ep2 = pe_ps.tile([P, 512], F32, tag="eps")
            for kt in range(nmt1):
                nc.tensor.matmul(
                    ep2[:],
                    w_ap(w2, kt),
                    hts[kt][:],
                    start=(kt == 0),
                    stop=(kt == nmt1 - 1),
                )
            relu_to(engines[2], out_ap, ep2[:], b_ap(b2, 0))

        def allgather_x(x_own_t, t):
            xnm = pxnm.tile([P, NODE_SLOTS], BF16, tag="xnm")
            for b in range(NODE_SLOTS // 512):
                tp = ptr.tile([P, 512], BF16, tag="xtr")
                for j in range(4):
                    c = b * 4 + j
                    nc.tensor.transpose(
                        tp[:, j * P : (j + 1) * P],
                        x_own_t[:, c * P : (c + 1) * P], ident,
                    )
                if b % 2 == 0:
                    nc.vector.tensor_copy(xnm[:, b * 512 : (b + 1) * 512], tp[:])
                else:
                    nc.scalar.copy(xnm[:, b * 512 : (b + 1) * 512], tp[:])
            nc.sync.dma_start(
                x_pad[t][:].rearrange("(c p) f -> p c f", p=P),
                xnm[:].rearrange("p (c f) -> p c f", f=P),
            )
            if knocc:
                nc.gpsimd.dma_start(x_full[t][0:NODE_SLOTS, :], x_pad[t][:])
            else:
                nc.gpsimd.collective_compute(
                    "AllGather",
                    mybir.AluOpType.bypass,
                    ins=[x_pad[t].opt()],
                    outs=[x_full[t].opt()],
                    replica_groups=[list(range(N_CORES))],
                )

        # ---------------- embed ----------------
        pooled_sb = [
            pout.tile([1, NODE_SLOTS], F32, name=f"pool{t}", tag=f"pool{t}")
            for t in range(NUM_FINE)
        ]
        for t in range(NUM_FINE):
            nc.gpsimd.memset(pooled_sb[t][:], 0.0)

        # node embed
        x_own = pxo.tile([P, NODE_SLOTS], BF16, tag="xo")
        for nch in range(n_nchunk):
            nt = pml.tile([P, 512], BF16, tag="in512")
            nc.sync.dma_start(nt[:], nodes_T[:, nch * 512 : (nch + 1) * 512])
            mlp2([nt[:]], "Wn1", "bn1", "Wn2", "bn2",
                 x_own[:, nch * 512 : (nch + 1) * 512])

        allgather_x(x_own, 0)

        # edge embed -> efeat (SBUF resident)
        for mc in range(n_macro):
            et = pml.tile([P, 512], BF16, tag="in512")
            nc.sync.dma_start(et[:], edges_T[:, mc * 512 : (mc + 1) * 512])
            mlp2([et[:]], "We1", "be1", "We2", "be2",
                 efeat[:, mc * 512 : (mc + 1) * 512])

        # ---------------- fine iterations ----------------
        for t in range(NUM_FINE):
            xf = x_full[t]
            agg_sb = pagg.tile([P, NODE_SLOTS], BF16, tag="agg")
            g_s = None
            agg_ps = None
            yrt = None

            for mc in range(n_macro):
                wi, mcw = divmod(mc, n_macro_w)
                sl = slice(mc * 512, (mc + 1) * 512)

                # sender gather (bf16 rows). transpose mode lands feature-major
                # [128, GB]; plain mode lands row-major [128, GB/128, 128] and
                # is PE-transposed per chunk below.
                gi = mc % (GB // 512)
                if gi == 0:
                    shape = [P, 1, GB] if GATHER_T else [P, GB // P, P]
                    g_s = pg.tile(shape, BF16, tag="gs")
                    i0 = mc * 512 // 16
                    if os.environ.get("KNOGATHER") == "1":
                        fake = xf[0:GB, :].rearrange("(c p) f -> p c f", p=P)
                        nc.gpsimd.dma_start(
                            g_s[:].rearrange("p a b -> p (a b)")
                            .rearrange("p (c f) -> p c f", f=P),
                            fake)
                    else:
                        nc.gpsimd.dma_gather(
                            out_ap=g_s[:],
                            in_ap=xf[:],
                            idxs_ap=sidx[:, i0 : i0 + GB // 16],
                            num_idxs=GB,
                            num_idxs_reg=GB,
                            elem_size=P,
                            transpose=GATHER_T,
                            queue_num=(mc // (GB // 512)) % 4,
                        )

                if mcw == 0:
                    # window start: YrT_kt = x_win^T @ W1r  (one per 128-node half)
                    yrt = []
                    for kt in range(2):
                        yp = pyr_ps.tile([P, H], F32, tag="yr")
                        nsl = slice(wi * WIN + kt * P, wi * WIN + (kt + 1) * P)
                        nc.tensor.matmul(
                            yp[:], x_own[:, nsl], w_ap("Wed1r", 0),
                            start=True, stop=True,
                        )
                        ys = pyr.tile([P, H], BF16, tag=f"yrs{kt}")
                        nc.vector.tensor_copy(ys[:], yp[:])
                        yrt.append(ys)
                    agg_ps = pag_ps.tile([P, WIN], F32, tag="aggps")

                # receiver one-hot selectors [128 n, 512 e] per node-half
                rr = prr.tile([P, 512], BF16, tag="rr")
                nc.sync.dma_start(rr[:], rrel_rep[:, sl])
                sel_nm = []
                for kt in range(2):
                    s = psel.tile([P, 512], BF16, tag=f"selnm{kt}")
                    nc.vector.tensor_tensor(
                        out=s[:],
                        in0=rr[:],
                        in1=icol[:, kt : kt + 1].to_broadcast([P, 512]),
                        op=mybir.AluOpType.is_equal,
                    )
                    sel_nm.append(s)

                yr_terms = [
                    (lambda mt, _ys=yrt[kt]: _ys[:, mt * P : (mt + 1) * P],
                     sel_nm[kt][:])
                    for kt in range(2)
                ]

                if t < NUM_FINE - 1:
                    dst = efeat[:, sl]

                    def esrc(j, _mc=mc):
                        return efeat[:, _mc * 512 + j * P : _mc * 512 + (j + 1) * P]
                else:
                    e_new = pml.tile([P, 512], BF16, tag="eo")
                    dst = e_new[:]

                    def esrc(j, _e=e_new):
                        return _e[:, j * P : (j + 1) * P]

                if GATHER_T:
                    xs_ap = g_s[:, 0, gi * 512 : (gi + 1) * 512]
                else:
                    xsp = ptr.tile([P, 512], BF16, tag="xtr")
                    for j in range(4):
                        nc.tensor.transpose(
                            xsp[:, j * P : (j + 1) * P],
                            g_s[:, gi * 4 + j, :], ident,
                        )
                    xs = pml.tile([P, 512], BF16, tag="xssb")
                    nc.scalar.copy(xs[:], xsp[:])
                    xs_ap = xs[:]

                mlp2(
                    [xs_ap, efeat[:, sl]],
                    "Wed1", "bed1", "Wed2", "bed2", dst,
                    extra_l1=yr_terms,
                )

                # scatter: eT one-hot accumulate into window agg
                eTp = ptr.tile([P, 512], BF16, tag="etr")
                for j in range(4):
                    nc.tensor.transpose(eTp[:, j * P : (j + 1) * P], esrc(j), ident)
                eT = pml.tile([P, 512], BF16, tag="etsb")
                nc.scalar.copy(eT[:], eTp[:])
                for j in range(4):
                    sc = mc * 4 + j
                    selt = psel.tile([P, WIN], BF16, tag="selem")
                    nc.vector.tensor_tensor(
                        out=selt[:],
                        in0=rrelc[:, sc : sc + 1].to_broadcast([P, WIN]),
                        in1=iota_f,
                        op=mybir.AluOpType.is_equal,
                    )
                    nc.tensor.matmul(
                        agg_ps[:],
                        eT[:, j * P : (j + 1) * P],
                        selt[:],
                        start=(mcw == 0 and j == 0),
                        stop=(mcw == n_macro_w - 1 and j == 3),
                        skip_group_check=True,
                    )
                if mcw == n_macro_w - 1:
                    nc.vector.tensor_copy(
                        agg_sb[:, wi * WIN : (wi + 1) * WIN], agg_ps[:]
                    )

            # node MLP + pooled
            x_new = pxo.tile([P, NODE_SLOTS], BF16, tag="xo")
            for ncn in range(n_nchunk):
                sl = slice(ncn * 512, (ncn + 1) * 512)
                mlp2(
                    [x_own[:, sl], agg_sb[:, sl]],
                    "Wnd1", "bnd1", "Wnd2", "bnd2", x_new[:, sl],
                )
                pp = pml.tile([P, 512], F32, tag="prd")
                nc.gpsimd.partition_all_reduce(
                    pp[:], x_new[:, sl], channels=P,
                    reduce_op=bass.bass_isa.ReduceOp.add,
                )
                nc.vector.tensor_copy(pooled_sb[t][:, sl], pp[0:1, :])
            x_own = x_new
            if t < NUM_FINE - 1:
                allgather_x(x_own, t + 1)

        for t in range(NUM_FINE):
            nc.sync.dma_start(out_pooled[t : t + 1, :], pooled_sb[t][:])

    nc.compile()
    return nc


# ----------------------------------------------------------------------------
# Entry point
# ----------------------------------------------------------------------------

def _prepare(inputs):
    nodes = np.asarray(inputs["nodes"], dtype=np.float32)
    edges = np.asarray(inputs["edges"], dtype=np.float32)
    senders = np.asarray(inputs["senders"]).astype(np.int64)
    receivers = np.asarray(inputs["receivers"]).astype(np.int64)

    ws = {k: np.asarray(inputs[k]) for k in
          ["Wn1", "Wn2", "We1", "We2", "Wed1", "Wed2", "Wnd1", "Wnd2"]}
    bs = {k: np.asarray(inputs[k]) for k in
          ["bn1", "bn2", "be1", "be2", "bed1", "bed2", "bnd1", "bnd2"]}

    rslot = _pad_slot(receivers)
    counts = np.bincount(rslot // WIN, minlength=N_CORES * N_WIN)
    ep_win = int(math.ceil(counts.max() / 512) * 512)

    # Split Wed1 into [sender D | receiver D | edge D] k-tiles; the receiver
    # block is applied via the windowed Yr trick, so the matmul blob carries
    # sender (kt 0) and edge (kt 1) tiles under "Wed1" and the receiver block
    # separately under "Wed1r".
    Wed1 = np.asarray(ws.pop("Wed1"))
    ws2 = dict(ws)
    ws2["Wed1"] = np.concatenate([Wed1[0:D], Wed1[2 * D : 3 * D]], axis=0)
    ws2["Wed1r"] = Wed1[D : 2 * D]

    wblob, woffs = build_weight_blob(ws2)
    bblob, boffs = build_bias_blob(bs)

    in_maps = []
    for k in range(N_CORES):
        m = prepare_core(k, nodes, edges, senders, receivers, ep_win)
        m["wblob"] = wblob
        m["bblob"] = bblob
        in_maps.append(m)

    nc = build_program(ep_win, wblob.shape[1], bblob.shape[1], woffs, boffs)
    return nc, in_maps


def _assemble(results):
    out = np.zeros(NUM_FINE * N, dtype=np.float32)
    for k in range(N_CORES):
        pooled = results[k]["out_pooled"]
        for t in range(NUM_FINE):
            out[t * N + k * NODES_PER_CORE : t * N + (k + 1) * NODES_PER_CORE] = (
                pooled[t, :NODES_PER_CORE]
            )
    return out


def _run(inputs, trace=False):
    nc, in_maps = _prepare(inputs)
    res = run_bass_kernel_spmd(
        nc, in_maps, core_ids=list(range(N_CORES)), trace=trace
    )
    return _assemble(res.results), res


def kernel(**inputs):
    out, _ = _run(inputs, trace=False)
    return out
